# revision 1
# baseline (speedup 1.0000x reference)
"""Trainium2 Bass kernel for thresholded multi-head attention.

Computes, for x:[b,n,dim] with b=4, n=2048, dim=512, heads=8, dh=64:
    qkv = x @ Wqkv + bqkv ; split q,k,v per head
    dots = q k^T / sqrt(dh) ; attn = softmax(dots)
    attn = where(attn > 0.01, attn, 0) ; out = attn @ v
    return out @ Wout + bout

Sharding over 8 NeuronCores: core c handles batch b = c//2 and head group
g = c%2 (4 of the 8 heads), producing a partial output projection for its
batch; host sums the two partials per batch and adds bout.

Numerics: fp16 two-limb (hi+lo) matmuls for the qk projection and the
attention logits (error ~1e-5, threshold-flip free vs fp32); exp in fp32 on
the Scalar engine (softmax without max subtraction: logits have unit
variance so exp is fp32-safe); the softmax denominator Z is accumulated on
the Tensor engine from two 16-bit limbs of E (fp16 RNE copy + bf16
residual, |dZ|/Z ~ 3e-7); the attn>0.01 compare is then fp32-exact against
c = 0.01*Z via a one-pass custom DVE select op; masked weights and V go
through the PE in bf16. Measured vs the CPU fp32 reference: absmax error
5.2e-4 (0.4% of the output absmax), zero threshold flips.
"""
import os
import sys
import functools

import numpy as np

for _p in ("/opt/trn_rl_repo", "/root/.axon_site", "/root/.axon_site/_ro/trn_rl_repo"):
    if os.path.isdir(_p) and _p not in sys.path:
        sys.path.append(_p)

import ml_dtypes
from contextlib import ExitStack

import concourse.bass as bass
import concourse.bacc as bacc
import concourse.mybir as mybir
import concourse.tile as tile
from concourse import bass_utils

FP32 = mybir.dt.float32
FP16 = mybir.dt.float16
BF16 = mybir.dt.bfloat16
ALU = mybir.AluOpType
AFT = mybir.ActivationFunctionType

BF16_TRUNC_CORR = 1.0 + 2.0 ** -8  # legacy (unused)


def _register_mask_op():
    """One-pass masked keep: out = in0 if in1 < in0 else 0.

    Registered through the documented custom-DVE extension point
    (dve_ops.OPS); used with in0 = E (fp32) and in1 = broadcast threshold.
    """
    from concourse.dve_spec import Spec, Src0, Src1, Zero, select
    from concourse import dve_ops as dops

    name = "MASK_KEEP_GT_ANT"
    for op in dops.OPS:
        if op.name == name:
            return op
    op = dops.DveOp(
        name,
        Spec(
            body=select(Src1 < Src0, Src0, Zero),
            reference=lambda in0, in1, s0, s1, imm2: np.where(
                in1 < in0, in0, 0.0).astype(np.float32),
        ),
        subdim=False,
        uops_sha={"v3": "d86f8416d0d7b042", "v4": "f70e64aee8639ca3"},
    )
    dops.OPS.append(op)
    dops._SUB_OPCODE_FOR_NAME[name] = dops._CUSTOM_DVE_ROW_BASE + len(dops.OPS) - 1
    dops.CUSTOM_DVE_SPECS[name] = op.spec
    return op


MASK_OP = _register_mask_op()


def emit_core_kernel(ctx, tc, io, n=2048, dim=512, hc=4, dh=64, qch=512):
    """Emit one core's program. io: dict name -> bass.AP (DRAM).

    hc: heads on this core. qch: query-chunk (free-dim of S^T tiles).
    """
    nc = tc.nc
    inner = hc * dh                 # 256
    NT = n // 128                   # x row tiles / key chunks
    KC = n // 128
    SG = 2 if qch * 4 >= 2048 else 1  # key chunks per S/E tile (PSUM banks)
    KC2 = KC // SG                  # E-tiles per (h,qc)
    QC = n // qch
    DC = dim // 128                 # contraction chunks of dim
    MH = inner // 128               # m-tiles of qT (and of kT)
    MQK = 2 * MH                    # m-tiles of stacked [q;k]T
    scale = dh ** -0.5

    # ---------------- constants ----------------
    cpool = ctx.enter_context(tc.tile_pool(name="consts", bufs=1))
    ident = cpool.tile([128, 128], FP16, tag="ident", name="ident")
    nc.sync.dma_start(ident[:], io["ident"][:])
    wqk_h = []
    wqk_x = []
    wv_h = []
    for c in range(DC):
        t = cpool.tile([128, 2 * inner], FP16, tag=f"wqkh{c}", name=f"wqkh{c}")
        nc.sync.dma_start(t[:], io["wqk_h"][c * 128:(c + 1) * 128, :])
        wqk_h.append(t)
        t = cpool.tile([128, inner], FP16, tag=f"wvh{c}", name=f"wvh{c}")
        nc.sync.dma_start(t[:], io["wv_h"][c * 128:(c + 1) * 128, :])
        wv_h.append(t)
    for c in range(2 * DC):
        t = cpool.tile([128, 2 * inner], FP16, tag=f"wqkx{c}", name=f"wqkx{c}")
        nc.sync.dma_start(t[:], io["wqk_x"][c * 128:(c + 1) * 128, :])
        wqk_x.append(t)
    wout = []
    for m in range(MH):
        t = cpool.tile([128, dim], BF16, tag=f"wout{m}", name=f"wout{m}")
        nc.sync.dma_start(t[:], io["wout_b"][m * 128:(m + 1) * 128, :])
        wout.append(t)
    bqk = []
    for m in range(MQK):
        t = cpool.tile([128, 1], FP32, tag=f"bqk{m}", name=f"bqk{m}")
        nc.sync.dma_start(t[:], io["bqk"][m * 128:(m + 1) * 128, :])
        bqk.append(t)
    bv_row = cpool.tile([1, inner], FP32, tag="bv", name="bv_row")
    nc.sync.dma_start(bv_row[:], io["bv"][:])
    ones_col_bf = cpool.tile([128, 1], BF16, tag="ones_col", name="ones_col")
    nc.vector.memset(ones_col_bf[:], 1.0)
    ones_col_f16 = cpool.tile([128, 1], FP16, tag="ones_col16", name="ones_col16")
    nc.vector.memset(ones_col_f16[:], 1.0)
    ones_row_f = cpool.tile([1, 128], FP32, tag="ones_row", name="ones_row")
    nc.vector.memset(ones_row_f[:], 1.0)

    # persistent activations
    apool = ctx.enter_context(tc.tile_pool(name="acts", bufs=1))
    qkT_h = [apool.tile([128, n], FP16, tag=f"qkTh{m}", name=f"qkTh{m}") for m in range(MQK)]
    qkT_l = [apool.tile([128, n], FP16, tag=f"qkTl{m}", name=f"qkTl{m}") for m in range(MQK)]
    V_sb = [apool.tile([128, inner], BF16, tag=f"V{t}", name=f"V{t}") for t in range(NT)]
    attnT = [apool.tile([128, n], FP32, tag=f"attnT{m}", name=f"attnT{m}") for m in range(MH)]
    attnB = [apool.tile([128, n], BF16, tag=f"attnB{m}", name=f"attnB{m}") for m in range(MH)]

    # ---------------- phase A: x -> xT (hi/lo fp16) ----------------
    with tc.tile_pool(name="xT", bufs=1) as xtp:
        xTh = [xtp.tile([128, n], FP16, tag=f"xTh{c}", name=f"xTh{c}") for c in range(DC)]
        xTl = [xtp.tile([128, n], FP16, tag=f"xTl{c}", name=f"xTl{c}") for c in range(DC)]
        with tc.tile_pool(name="xin", bufs=4) as xip, \
             tc.tile_pool(name="psA", bufs=4, space="PSUM") as psA:
            for nt in range(NT):
                for src, dsts in (("xh", xTh), ("xl", xTl)):
                    xt = xip.tile([128, dim], FP16, tag="xin")
                    nc.sync.dma_start(xt[:], io[src][nt * 128:(nt + 1) * 128, :])
                    for c in range(DC):
                        ps = psA.tile([128, 128], FP16, tag="psA")
                        nc.tensor.transpose(ps[:], xt[:, c * 128:(c + 1) * 128],
                                            ident[:])
                        nc.vector.tensor_copy(
                            dsts[c][:, nt * 128:(nt + 1) * 128], ps[:])

        # ---------------- phase B: projections ----------------
        nqs = min(512, n)
        with tc.tile_pool(name="psB", bufs=4, space="PSUM") as psB:
            # qkT = (Wqk^T x^T) as hi+lo fp16, with bias
            for m in range(MQK):
                for nq in range(n // nqs):
                    ps = psB.tile([128, nqs], FP32, tag="psB")
                    sl = slice(nq * nqs, (nq + 1) * nqs)
                    msl = slice(m * 128, (m + 1) * 128)
                    for c in range(DC):
                        nc.tensor.matmul(ps[:], wqk_h[c][:, msl],
                                         xTh[c][:, sl],
                                         start=(c == 0), stop=False)
                    for c2 in range(2 * DC):
                        rhs = xTh[c2][:, sl] if c2 < DC else xTl[c2 - DC][:, sl]
                        nc.tensor.matmul(ps[:], wqk_x[c2][:, msl], rhs,
                                         start=False, stop=(c2 == 2 * DC - 1))
                    nc.vector.tensor_scalar(qkT_h[m][:, sl], ps[:], bqk[m][:],
                                            None, ALU.add)
                    nc.vector.scalar_tensor_tensor(
                        qkT_l[m][:, sl], ps[:], bqk[m][:], qkT_h[m][:, sl],
                        ALU.add, ALU.subtract)
            # V natural [n, inner] in bf16, bias via rank-1 ones
            for nt in range(NT):
                ps = psB.tile([128, inner], FP32, tag="psBv")
                tsl = slice(nt * 128, (nt + 1) * 128)
                for c in range(DC):
                    nc.tensor.matmul(ps[:], xTh[c][:, tsl], wv_h[c][:],
                                     start=(c == 0), stop=False)
                nc.tensor.matmul(ps[:], ones_row_f[:], bv_row[:],
                                 start=False, stop=True)
                nc.vector.tensor_copy(V_sb[nt][:], ps[:])

    # ---------------- phase C: attention ----------------
    # Z is accumulated on the PE from two 16-bit limbs of E: an fp16 RNE
    # copy (11 bits, DVE cast at 2x) plus a bf16 residual (8 more bits,
    # computed on GPSIMD) -> |dZ|/Z ~ 3e-7, so the attn>0.01 compare sees
    # an effectively fp32-exact threshold.
    #
    # Software pipeline: iteration i+1's S/exp/limb/Z work is emitted
    # before iteration i's mask/PV tail so the PE's in-order queue never
    # stalls on DVE-produced mask tiles; Z matmuls are skewed two S-tiles
    # behind the exp that feeds them for the same reason.
    with tc.tile_pool(name="psS", bufs=2, space="PSUM") as psS, \
         tc.tile_pool(name="psZZ", bufs=2, space="PSUM") as psZZp, \
         tc.tile_pool(name="psCB", bufs=1, space="PSUM") as psCBp, \
         tc.tile_pool(name="psO", bufs=1, space="PSUM") as psOp, \
         tc.tile_pool(name="Epool", bufs=2 * KC2, space="SBUF") as Ep, \
         tc.tile_pool(name="limb", bufs=6) as lp, \
         tc.tile_pool(name="mp", bufs=2 * KC2, space="SBUF") as mp, \
         tc.tile_pool(name="crow", bufs=1) as crp:

        def flush_z2(jobs):
            """Residual-limb Z matmuls; deferred a full pipeline stage so the
            PE never waits on the GPSIMD-produced Er tiles."""
            for kt_, Er_, psZZ_ in jobs:
                for j in range(SG):
                    jsl = slice(j * qch, (j + 1) * qch)
                    nc.tensor.matmul(psZZ_[32:33, :], ones_col_bf[:],
                                     Er_[:, jsl],
                                     start=(kt_ == 0 and j == 0),
                                     stop=(kt_ == KC2 - 1 and j == SG - 1))
            jobs.clear()

        def stage_a(h, qc, z2_prev, bhead=None):
            """S^T matmuls, exp, Z limbs, Z accumulation for one (h, qc)."""
            mq, rq = h // 2, 64 * (h % 2)
            mk = MH + h // 2
            qsl_h = slice(rq, rq + 64)
            qsl = slice(qc * qch, (qc + 1) * qch)
            q_hi = qkT_h[mq][qsl_h, qsl]
            q_lo = qkT_l[mq][qsl_h, qsl]
            psZZ = psZZp.tile([33, qch], FP32, tag="ZZ")
            E_tiles = []
            pending_z = []
            z2_jobs = []

            def flush_z1(limit):
                while len(pending_z) > limit:
                    kt_, Eh_ = pending_z.pop(0)
                    for j in range(SG):
                        jsl = slice(j * qch, (j + 1) * qch)
                        nc.tensor.matmul(psZZ[0:1, :], ones_col_f16[:],
                                         Eh_[:, jsl],
                                         start=(kt_ == 0 and j == 0),
                                         stop=(kt_ == KC2 - 1 and j == SG - 1))

            for kt in range(KC2):
                ps = psS.tile([128, SG * qch], FP32, tag="S")
                for j in range(SG):
                    kc = SG * kt + j
                    ksl = slice(kc * 128, (kc + 1) * 128)
                    out = ps[:, j * qch:(j + 1) * qch]
                    k_hi = qkT_h[mk][qsl_h, ksl]
                    k_lo = qkT_l[mk][qsl_h, ksl]
                    nc.tensor.matmul(out, k_hi, q_hi, start=True, stop=False)
                    nc.tensor.matmul(out, k_lo, q_hi, start=False, stop=False)
                    nc.tensor.matmul(out, k_hi, q_lo, start=False, stop=True)
                if kt == 1 and z2_prev:
                    flush_z2(z2_prev)
                    if bhead is not None:
                        bhead()
                Et = Ep.tile([128, SG * qch], FP32, tag="E")
                nc.scalar.activation(Et[:], ps[:], AFT.Exp, scale=scale)
                E_tiles.append(Et)
                # two 16-bit limbs of E for the exact-Z matmuls
                Eh = lp.tile([128, SG * qch], FP16, tag="Eh")
                nc.vector.tensor_copy(Eh[:], Et[:])
                Er = lp.tile([128, SG * qch], BF16, tag="Er", bufs=10)
                nc.gpsimd.tensor_tensor(Er[:], Et[:], Eh[:], ALU.subtract)
                pending_z.append((kt, Eh))
                z2_jobs.append((kt, Er, psZZ))
                flush_z1(2)
            flush_z1(0)
            return E_tiles, psZZ, z2_jobs

        def stage_b_head(state):
            """Z finalize + threshold broadcast; emitted early (inside the
            next iteration's stage_a) so the PE/DVE see it promptly."""
            E_tiles, psZZ, _ = state
            z2_row = crp.tile([1, qch], FP32, tag="z2row")
            nc.scalar.activation(z2_row[:], psZZ[32:33, :], AFT.Copy)
            z_row = crp.tile([1, qch], FP32, tag="zrow")
            nc.vector.scalar_tensor_tensor(z_row[:], psZZ[0:1, :], 0.0,
                                           z2_row[:], ALU.add, ALU.add)
            c_row = crp.tile([1, qch], FP32, tag="crow")
            nc.vector.tensor_scalar(c_row[:], z_row[:], 0.01, None, ALU.mult)
            r_row = crp.tile([1, qch], FP32, tag="rrow")
            nc.vector.reciprocal_approx_fast(out=r_row[:], in_=z_row[:])
            psCB = psCBp.tile([128, qch], FP32, tag="CB")
            nc.tensor.matmul(psCB[:], ones_row_f[:], c_row[:],
                             start=True, stop=True)
            return psCB, r_row

        def stage_b(h, qc, state, head):
            """masks, PV, 1/Z scale for one (h, qc)."""
            E_tiles, psZZ, _ = state
            psCB, r_row = head
            mq, rq = h // 2, 64 * (h % 2)
            qsl_h = slice(rq, rq + 64)
            qsl = slice(qc * qch, (qc + 1) * qch)
            P_tiles = []
            for kt in range(KC2):
                Et = E_tiles[kt]
                for j in range(SG):
                    esl = Et[:, j * qch:(j + 1) * qch]
                    Pt = mp.tile([128, qch], BF16, tag="P")
                    nc.vector._custom_dve(MASK_OP, out=Pt[:], in0=esl,
                                          in1=psCB[:])
                    P_tiles.append(Pt)
            psO = psOp.tile([64, qch], FP32, tag="O")
            for kc in range(KC):
                nc.tensor.matmul(psO[:], V_sb[kc][:, h * dh:(h + 1) * dh],
                                 P_tiles[kc][:],
                                 start=(kc == 0), stop=(kc == KC - 1))
            nc.scalar.activation(attnT[mq][qsl_h, qsl], psO[:], AFT.Copy)
            # scale by 1/Z: broadcast r over the 64 head dims, multiply
            psR = psZZp.tile([64, qch], FP32, tag="ZZ")
            nc.tensor.matmul(psR[:], ones_row_f[:, :64], r_row[:],
                             start=True, stop=True)
            nc.vector.tensor_tensor(attnB[mq][qsl_h, qsl],
                                    attnT[mq][qsl_h, qsl], psR[:],
                                    ALU.mult)

        order = [(h, qc) for h in range(hc) for qc in range(QC)]
        prev = None
        z2_prev = []
        head_box = {}
        for hq in order:
            pstate = prev[1] if prev is not None else None
            bhead = (lambda s=pstate: head_box.__setitem__("h", stage_b_head(s))) \
                if pstate is not None else None
            state = stage_a(hq[0], hq[1], z2_prev, bhead)
            z2_prev = state[2]
            if prev is not None:
                stage_b(prev[0][0], prev[0][1], prev[1], head_box.pop("h"))
            prev = (hq, state)
        flush_z2(z2_prev)
        head = stage_b_head(prev[1])
        stage_b(prev[0][0], prev[0][1], prev[1], head)

    # ---------------- phase E: output projection ----------------
    with tc.tile_pool(name="psE", bufs=4, space="PSUM") as psE, \
         tc.tile_pool(name="ostage", bufs=4) as osp:
        for nt in range(NT):
            ps = psE.tile([128, dim], FP32, tag="psE")
            tsl = slice(nt * 128, (nt + 1) * 128)
            for m in range(MH):
                nc.tensor.matmul(ps[:], attnB[m][:, tsl], wout[m][:],
                                 start=(m == 0), stop=(m == MH - 1))
            ot = osp.tile([128, dim], FP32, tag="ostage")
            eng = nc.vector if nt % 2 == 0 else nc.scalar
            if eng is nc.scalar:
                nc.scalar.activation(ot[:], ps[:], AFT.Copy)
            else:
                nc.vector.tensor_copy(ot[:], ps[:])
            nc.sync.dma_start(io["out"][tsl, :], ot[:])


def build_program(n=2048, dim=512, hc=4, dh=64, qch=512):
    nc = bacc.Bacc(trn_type="TRN2", target_bir_lowering=False, debug=False)
    inner = hc * dh
    io = {}

    def din(name, shape, dt):
        io[name] = nc.dram_tensor(name, shape, dt, kind="ExternalInput").ap()

    din("xh", [n, dim], FP16)
    din("xl", [n, dim], FP16)
    din("wqk_h", [dim, 2 * inner], FP16)
    din("wqk_x", [2 * dim, 2 * inner], FP16)
    din("wv_h", [dim, inner], FP16)
    din("bqk", [2 * inner, 1], FP32)
    din("bv", [1, inner], FP32)
    din("wout_b", [inner, dim], BF16)
    din("ident", [128, 128], FP16)
    io["out"] = nc.dram_tensor("out", [n, dim], FP32, kind="ExternalOutput").ap()

    with tile.TileContext(nc) as tc:
        with ExitStack() as ctx:
            emit_core_kernel(ctx, tc, io, n=n, dim=dim, hc=hc, dh=dh, qch=qch)
    nc.compile()
    return nc


def make_core_inputs(x_b, Wq, Wk, Wv, bq, bk, bv, Wout_g, n=2048, dim=512,
                     hc=4, dh=64):
    """Host-side prep of one core's input dict (numpy, correct dtypes)."""
    f16 = np.float16
    inner = hc * dh
    xh = x_b.astype(f16)
    xl = (x_b - xh.astype(np.float32)).astype(f16)
    wqk = np.concatenate([Wq, Wk], axis=1)              # [dim, 2*inner]
    wqk_hi = wqk.astype(f16)
    wqk_lo = (wqk - wqk_hi.astype(np.float32)).astype(f16)
    wqk_x = np.concatenate([wqk_lo, wqk_hi], axis=0)    # [2*dim, 2*inner]
    return {
        "xh": xh, "xl": xl,
        "wqk_h": wqk_hi, "wqk_x": wqk_x,
        "wv_h": Wv.astype(f16),
        "bqk": np.concatenate([bq, bk]).reshape(2 * inner, 1).astype(np.float32),
        "bv": bv.reshape(1, inner).astype(np.float32),
        "wout_b": Wout_g.astype(ml_dtypes.bfloat16),
        "ident": np.eye(128, dtype=f16),
    }


@functools.lru_cache(maxsize=1)
def _cached_program():
    return build_program()


def kernel(x, Wqkv, bqkv, Wout, bout):
    x = np.asarray(x, dtype=np.float32)
    Wqkv = np.asarray(Wqkv, dtype=np.float32)
    bqkv = np.asarray(bqkv, dtype=np.float32)
    Wout = np.asarray(Wout, dtype=np.float32)
    bout = np.asarray(bout, dtype=np.float32)

    b, n, dim = x.shape
    H, dh = 8, 64
    inner = H * dh
    hc = 4  # heads per core
    Wq, Wk, Wv = Wqkv[:, :inner], Wqkv[:, inner:2 * inner], Wqkv[:, 2 * inner:]
    bq, bk, bv = bqkv[:inner], bqkv[inner:2 * inner], bqkv[2 * inner:]

    in_maps = []
    for c in range(8):
        bb, g = c // 2, c % 2
        hsl = slice(g * hc * dh, (g + 1) * hc * dh)
        in_maps.append(make_core_inputs(
            x[bb], Wq[:, hsl], Wk[:, hsl], Wv[:, hsl],
            bq[hsl], bk[hsl], bv[hsl], Wout[hsl, :],
            n=n, dim=dim, hc=hc, dh=dh))

    nc = _cached_program()
    res = bass_utils.run_bass_kernel_spmd(nc, in_maps, core_ids=list(range(8)))
    global LAST_RESULTS
    LAST_RESULTS = res
    out = np.empty((b, n, dim), dtype=np.float32)
    for bb in range(b):
        out[bb] = res.results[2 * bb]["out"] + res.results[2 * bb + 1]["out"] \
            + bout
    return out



# revision 15
# speedup vs baseline: 1.0632x; 1.0632x over previous
"""Trainium2 Bass kernel for thresholded multi-head attention.

Computes, for x:[b,n,dim] with b=4, n=2048, dim=512, heads=8, dh=64:
    qkv = x @ Wqkv + bqkv ; split q,k,v per head
    dots = q k^T / sqrt(dh) ; attn = softmax(dots)
    attn = where(attn > 0.01, attn, 0) ; out = attn @ v
    return out @ Wout + bout

Sharding over 8 NeuronCores: core c handles batch b = c//2 and head group
g = c%2 (4 of the 8 heads), producing a partial output projection for its
batch; host sums the two partials per batch and adds bout.

Numerics (zero threshold flips required: min |w/0.01-1| on this data is
1.5e-6 and a single flip contributes ~1e-2 absmax vs a 2.8e-3 budget):
  - q/k projection: 3-limb fp16 (Wh*xh + Wl*xh + Wh*xl), fp32 PSUM.
  - S^T logits: FULL fp32-exact product via 2 K-stacked fp16 matmuls:
    [k_hi;k_lo]^T [q_hi;q_hi] + [k_hi;k_lo]^T [q_lo;q_lo]. Same PE cost
    as a 3-limb scheme but exactly (k_hi+k_lo)*(q_hi+q_lo).
  - exp in fp32 on Scalar (no max-subtraction; unit-variance logits).
  - Z accumulated on the PE from fp32 E read as float32r (full-rate
    moving operand at N=512); threshold c = 0.01*Z broadcast across
    partitions on GPSIMD (bit-exact fp32 copy), masked via the one-pass
    custom DVE select; PV in fp16; 1/Z via fast reciprocal broadcast.
"""
import os
import sys
import functools

import numpy as np

for _p in ("/opt/trn_rl_repo", "/root/.axon_site", "/root/.axon_site/_ro/trn_rl_repo"):
    if os.path.isdir(_p) and _p not in sys.path:
        sys.path.append(_p)

from contextlib import ExitStack

import concourse.bass as bass
import concourse.bacc as bacc
import concourse.mybir as mybir
import concourse.tile as tile
from concourse import bass_utils

FP32 = mybir.dt.float32
FP16 = mybir.dt.float16
F32R = mybir.dt.float32r
ALU = mybir.AluOpType
AFT = mybir.ActivationFunctionType


def _register_mask_op():
    """One-pass masked keep: out = in0 if in1 < in0 else 0.

    Registered through the documented custom-DVE extension point
    (dve_ops.OPS); used with in0 = E (fp32) and in1 = broadcast threshold.
    """
    from concourse.dve_spec import Spec, Src0, Src1, Zero, select
    from concourse import dve_ops as dops

    name = "MASK_KEEP_GT_ANT"
    for op in dops.OPS:
        if op.name == name:
            return op
    op = dops.DveOp(
        name,
        Spec(
            body=select(Src1 < Src0, Src0, Zero),
            reference=lambda in0, in1, s0, s1, imm2: np.where(
                in1 < in0, in0, 0.0).astype(np.float32),
        ),
        subdim=False,
        uops_sha={"v3": "d86f8416d0d7b042", "v4": "f70e64aee8639ca3"},
    )
    dops.OPS.append(op)
    dops._SUB_OPCODE_FOR_NAME[name] = dops._CUSTOM_DVE_ROW_BASE + len(dops.OPS) - 1
    dops.CUSTOM_DVE_SPECS[name] = op.spec
    return op


def _register_mask_scaled_op():
    """One-pass scaled masked keep: out = in0 if in1*imm2 < in0 else 0.

    Used with in0 = E (fp32), in1 = Z broadcast across partitions and
    imm2 = 0.01: keeps exactly the attention weights above the threshold,
    with the 0.01*Z product evaluated in fp32 inside the DVE."""
    from concourse.dve_spec import Spec, Src0, Src1, Zero, C2, select
    from concourse import dve_ops as dops

    name = "MASK_SCALED_GT_ANT"
    for op in dops.OPS:
        if op.name == name:
            return op
    op = dops.DveOp(
        name,
        Spec(
            body=select(Src1 * C2 < Src0, Src0, Zero),
            reference=lambda in0, in1, s0, s1, imm2: np.where(
                in1 * imm2 < in0, in0, 0.0).astype(np.float32),
        ),
        subdim=False,
        uops_sha={"v3": "e08cfe460da476fc", "v4": "03d53b4494ee5f42"},
    )
    dops.OPS.append(op)
    dops._SUB_OPCODE_FOR_NAME[name] = dops._CUSTOM_DVE_ROW_BASE + len(dops.OPS) - 1
    dops.CUSTOM_DVE_SPECS[name] = op.spec
    return op


MASK_OP = _register_mask_op()
MASK2_OP = _register_mask_scaled_op()


def emit_core_kernel(ctx, tc, io, n=2048, dim=512, hc=4, dh=64, qch=512):
    """Emit one core's program. io: dict name -> bass.AP (DRAM)."""
    nc = tc.nc
    inner = hc * dh                 # 256
    NT = n // 128                   # row tiles of n
    KT = n // 128                   # key tiles per (h, qc)
    QC = n // qch                   # query chunks per head
    DC = dim // 128                 # contraction chunks of dim
    MQK = 2 * inner // 128          # m-tiles of stacked [q;k] dims (4)
    MH = inner // 128               # m-tiles of attn-out dims (2)
    NQ = n // qch                   # 512-wide n chunks in phase B
    scale = dh ** -0.5

    # ---------------- constants ----------------
    cpool = ctx.enter_context(tc.tile_pool(name="consts", bufs=1))
    wqk_h = []
    wqk_x = []
    wv_h = []
    for c in range(DC):
        t = cpool.tile([128, 2 * inner], FP16, tag=f"wqkh{c}", name=f"wqkh{c}")
        nc.sync.dma_start(t[:], io["wqk_h"][c * 128:(c + 1) * 128, :])
        wqk_h.append(t)
        t = cpool.tile([128, inner], FP16, tag=f"wvh{c}", name=f"wvh{c}")
        nc.sync.dma_start(t[:], io["wv_h"][c * 128:(c + 1) * 128, :])
        wv_h.append(t)
    for c in range(2 * DC):
        t = cpool.tile([128, 2 * inner], FP16, tag=f"wqkx{c}", name=f"wqkx{c}")
        nc.sync.dma_start(t[:], io["wqk_x"][c * 128:(c + 1) * 128, :])
        wqk_x.append(t)
    wout = []
    for m in range(MH):
        t = cpool.tile([128, dim], FP16, tag=f"wout{m}", name=f"wout{m}")
        nc.sync.dma_start(t[:], io["wout_b"][m * 128:(m + 1) * 128, :])
        wout.append(t)
    # per-head biases, duplicated across both 64-partition halves so every
    # engine op reads its bias at the same start partition as its output
    bq2 = []
    bk2 = []
    for h in range(hc):
        t = cpool.tile([128, 1], FP32, tag=f"bq2{h}", name=f"bq2{h}")
        nc.sync.dma_start(t[:], io["bqk2"][h * 128:(h + 1) * 128, :])
        bq2.append(t)
        t = cpool.tile([128, 1], FP32, tag=f"bk2{h}", name=f"bk2{h}")
        nc.sync.dma_start(t[:], io["bqk2"][(hc + h) * 128:(hc + h + 1) * 128, :])
        bk2.append(t)
    bv_row = cpool.tile([1, inner], FP16, tag="bv", name="bv_row")
    nc.sync.dma_start(bv_row[:], io["bv"][:])
    ones_row16 = cpool.tile([1, 128], FP16, tag="ones_row16", name="ones_row16")
    nc.vector.memset(ones_row16[:], 1.0)

    # persistent activations
    apool = ctx.enter_context(tc.tile_pool(name="acts", bufs=1))
    qhh = [apool.tile([128, n], FP16, tag=f"qhh{h}", name=f"qhh{h}") for h in range(hc)]
    qll = [apool.tile([128, n], FP16, tag=f"qll{h}", name=f"qll{h}") for h in range(hc)]
    kstk = [apool.tile([128, n], FP16, tag=f"kstk{h}", name=f"kstk{h}") for h in range(hc)]
    V_sb = [apool.tile([128, inner], FP16, tag=f"V{t}", name=f"V{t}") for t in range(NT)]
    attnB = [apool.tile([128, n], FP16, tag=f"attnB{m}", name=f"attnB{m}") for m in range(MH)]

    # ---------------- phase B: projections (xT limbs DMA'd pre-transposed) --
    with tc.tile_pool(name="xT", bufs=1) as xtp:
        xTh = [xtp.tile([128, n], FP16, tag=f"xTh{c}", name=f"xTh{c}") for c in range(DC)]
        xTl = [xtp.tile([128, n], FP16, tag=f"xTl{c}", name=f"xTl{c}") for c in range(DC)]
        for c in range(DC):
            nc.sync.dma_start(xTh[c][:], io["xt_h"][c * 128:(c + 1) * 128, :])
            nc.sync.dma_start(xTl[c][:], io["xt_l"][c * 128:(c + 1) * 128, :])

        with tc.tile_pool(name="psB", bufs=4, space="PSUM") as psB:
            # qkT = (Wqk^T x^T) -> per-head stacked limb tiles, with bias
            for m in range(MQK):
                msl = slice(m * 128, (m + 1) * 128)
                for nq in range(NQ):
                    sl = slice(nq * qch, (nq + 1) * qch)
                    ps = psB.tile([128, qch], FP32, tag="psB")
                    for c in range(DC):
                        nc.tensor.matmul(ps[:], wqk_h[c][:, msl], xTh[c][:, sl],
                                         start=(c == 0), stop=False)
                    for c2 in range(2 * DC):
                        rhs = xTh[c2][:, sl] if c2 < DC else xTl[c2 - DC][:, sl]
                        nc.tensor.matmul(ps[:], wqk_x[c2][:, msl], rhs,
                                         start=False, stop=(c2 == 2 * DC - 1))
                    is_q = m < MH
                    for hi_half in range(2):      # which head within the m-tile
                        h = 2 * (m % MH) + hi_half
                        pr = slice(64 * hi_half, 64 * hi_half + 64)
                        if is_q:
                            # hi / lo limbs into the low halves; the high
                            # halves are duplicated by SBUF->SBUF DMA below
                            nc.scalar.activation(qhh[h][0:64, sl], ps[pr, :],
                                                 AFT.Identity,
                                                 bias=bq2[h][0:64])
                            nc.vector.scalar_tensor_tensor(
                                qll[h][0:64, sl], ps[pr, :], bq2[h][0:64],
                                qhh[h][0:64, sl], ALU.add, ALU.subtract)
                        else:
                            # kstk = [k_hi; k_lo]: hi into rows 0:64, DMA-dup
                            # into 64:128, then overwrite in place with the
                            # residual (in1 start partition == out's).
                            nc.scalar.activation(kstk[h][0:64, sl], ps[pr, :],
                                                 AFT.Identity,
                                                 bias=bk2[h][0:64])
                            nc.sync.dma_start(kstk[h][64:128, sl],
                                              kstk[h][0:64, sl])
                            nc.vector.scalar_tensor_tensor(
                                kstk[h][64:128, sl], ps[pr, :], bk2[h][64:128],
                                kstk[h][64:128, sl], ALU.add, ALU.subtract)
            # duplicate q limb tiles into their high halves
            for h in range(hc):
                nc.sync.dma_start(qhh[h][64:128, :], qhh[h][0:64, :])
                nc.sync.dma_start(qll[h][64:128, :], qll[h][0:64, :])
            # V natural [n, inner] in fp16, bias via rank-1 ones
            for nt in range(NT):
                tsl = slice(nt * 128, (nt + 1) * 128)
                psv = psB.tile([128, inner], FP32, tag="psV")
                for c in range(DC):
                    nc.tensor.matmul(psv[:], xTh[c][:, tsl], wv_h[c][:],
                                     start=(c == 0), stop=False)
                nc.tensor.matmul(psv[:], ones_row16[:], bv_row[:],
                                 start=False, stop=True)
                nc.scalar.activation(V_sb[nt][:], psv[:], AFT.Copy)

    # ---------------- phase C: attention ----------------
    # Software pipeline, 2-deep: iteration i emits S/exp; i-1 emits the
    # Z-sum tree, the partition all-reduce and masks; i-2 emits PV + 1/Z
    # scale. The PE stream is [S(i) | PV(i-2)] and never waits on
    # DVE/GPSIMD-produced tiles from the current iteration. Z is summed
    # off the PE entirely: an fp32 add tree split across GPSIMD and DVE,
    # collapsed across partitions by GPSIMD's partition_all_reduce, which
    # leaves Z broadcast on all 128 partitions for the threshold compare.
    ST = KT // 2                    # S/E tiles per iteration (2 key-tiles each)
    with tc.tile_pool(name="psS", bufs=3, space="PSUM") as psSp, \
         tc.tile_pool(name="psO", bufs=2, space="PSUM") as psOp, \
         tc.tile_pool(name="Epool", bufs=ST + 4, space="SBUF") as Ep, \
         tc.tile_pool(name="Ppool", bufs=ST + 2, space="SBUF") as Pp, \
         tc.tile_pool(name="zsum", bufs=1) as zsp, \
         tc.tile_pool(name="zb", bufs=2) as zbp:

        def stage_S(h, qc):
            """S^T matmuls (fp32-exact via K-stacked fp16 limb pairs) + exp."""
            qsl = slice(qc * qch, (qc + 1) * qch)
            E_tiles = []
            for t in range(ST):
                ps = psSp.tile([128, 2 * qch], FP32, tag="S")
                for j in range(2):
                    ksl = slice((2 * t + j) * 128, (2 * t + j + 1) * 128)
                    out = ps[:, j * qch:(j + 1) * qch]
                    nc.tensor.matmul(out, kstk[h][:, ksl], qhh[h][:, qsl],
                                     start=True, stop=False)
                    nc.tensor.matmul(out, kstk[h][:, ksl], qll[h][:, qsl],
                                     start=False, stop=True)
                Et = Ep.tile([128, 2 * qch], FP32, tag="E")
                nc.scalar.activation(Et[:], ps[:], AFT.Exp, scale=scale)
                E_tiles.append(Et)
            return E_tiles

        def stage_Z(state):
            """Z-sum: fp32 add tree (GPSIMD+DVE, in-place) + partition
            all-reduce, leaving Z broadcast on all 128 partitions."""
            E = state["E"]
            ta = zsp.tile([128, 2 * qch], FP32, tag="zta")
            nc.gpsimd.tensor_tensor(ta[:], E[0][:], E[1][:], ALU.add)
            tb = zsp.tile([128, 2 * qch], FP32, tag="ztb")
            nc.gpsimd.tensor_tensor(tb[:], E[2][:], E[3][:], ALU.add)
            nc.gpsimd.tensor_tensor(ta[:], ta[:], tb[:], ALU.add)
            tc_ = zsp.tile([128, 2 * qch], FP32, tag="ztc")
            nc.gpsimd.tensor_tensor(tc_[:], E[4][:], E[5][:], ALU.add)
            td = zsp.tile([128, 2 * qch], FP32, tag="ztd")
            nc.vector.tensor_tensor(td[:], E[6][:], E[7][:], ALU.add)
            nc.vector.tensor_tensor(tc_[:], tc_[:], td[:], ALU.add)
            nc.vector.tensor_tensor(ta[:], ta[:], tc_[:], ALU.add)
            zpre = zsp.tile([128, qch], FP32, tag="zpre")
            nc.vector.tensor_tensor(zpre[:], ta[:, 0:qch], ta[:, qch:2 * qch],
                                    ALU.add)
            Zb = zbp.tile([128, qch], FP32, tag="Zb")
            nc.gpsimd.partition_all_reduce(Zb[:], zpre[:], 128,
                                           bass.bass_isa.ReduceOp.add)
            state["Zb"] = Zb

        def stage_mask(state):
            """1/Z + thresholded keep (0.01*Z folded into the DVE op)."""
            Zb = state["Zb"]
            rb = zbp.tile([128, qch], FP32, tag="rb")
            nc.vector.reciprocal_approx_fast(out=rb[:], in_=Zb[:])
            state["rb"] = rb
            P_tiles = []
            for t in range(ST):
                Pt = Pp.tile([128, 2 * qch], FP16, tag="P")
                for j in range(2):
                    jsl = slice(j * qch, (j + 1) * qch)
                    nc.vector._custom_dve(MASK2_OP, out=Pt[:, jsl],
                                          in0=state["E"][t][:, jsl],
                                          in1=Zb[:], imm2=0.01)
                P_tiles.append(Pt)
            state["P"] = P_tiles

        def stage_PV(state):
            """PV accumulation + 1/Z scale into attnB."""
            h, qc = state["hq"]
            qsl = slice(qc * qch, (qc + 1) * qch)
            hsl = slice(h * dh, (h + 1) * dh)
            mq, rq = h // 2, 64 * (h % 2)
            psO = psOp.tile([64, qch], FP32, tag="O")
            for t in range(ST):
                for j in range(2):
                    kt = 2 * t + j
                    nc.tensor.matmul(psO[:], V_sb[kt][:, hsl],
                                     state["P"][t][:, j * qch:(j + 1) * qch],
                                     start=(kt == 0), stop=(kt == KT - 1))
            nc.vector.tensor_tensor(attnB[mq][rq:rq + 64, qsl], psO[:],
                                    state["rb"][rq:rq + 64, :], ALU.mult)

        # 1-deep skew: PE stream per iteration is [S(i) | PV(i-1)]; the
        # DVE/GPSIMD Z-tree and masks of iteration i chase exp(i) and
        # complete during PV(i-1)/S(i+1).
        iters = [(h, qc) for h in range(hc) for qc in range(QC)]
        states = []
        for i, (h, qc) in enumerate(iters):
            states.append({"hq": (h, qc), "E": stage_S(h, qc)})
            stage_Z(states[i])
            stage_mask(states[i])
            if i >= 1:
                stage_PV(states[i - 1])
                states[i - 1] = None
        stage_PV(states[-1])

    # ---------------- phase E: output projection ----------------
    with tc.tile_pool(name="psE", bufs=4, space="PSUM") as psE, \
         tc.tile_pool(name="ostage", bufs=4) as osp:
        for nt in range(NT):
            tsl = slice(nt * 128, (nt + 1) * 128)
            ps = psE.tile([128, dim], FP32, tag="psE")
            for m in range(MH):
                nc.tensor.matmul(ps[:], attnB[m][:, tsl], wout[m][:],
                                 start=(m == 0), stop=(m == MH - 1))
            ot = osp.tile([128, dim], FP32, tag="ostage")
            if nt % 2 == 0:
                nc.vector.tensor_copy(ot[:], ps[:])
            else:
                nc.scalar.activation(ot[:], ps[:], AFT.Copy)
            nc.sync.dma_start(io["out"][tsl, :], ot[:])


def build_program(n=2048, dim=512, hc=4, dh=64, qch=512):
    nc = bacc.Bacc(trn_type="TRN2", target_bir_lowering=False, debug=False)
    inner = hc * dh
    io = {}

    def din(name, shape, dt):
        io[name] = nc.dram_tensor(name, shape, dt, kind="ExternalInput").ap()

    din("xt_h", [dim, n], FP16)
    din("xt_l", [dim, n], FP16)
    din("wqk_h", [dim, 2 * inner], FP16)
    din("wqk_x", [2 * dim, 2 * inner], FP16)
    din("wv_h", [dim, inner], FP16)
    din("bqk2", [2 * hc * 128, 1], FP32)
    din("bv", [1, inner], FP16)
    din("wout_b", [inner, dim], FP16)
    io["out"] = nc.dram_tensor("out", [n, dim], FP32, kind="ExternalOutput").ap()

    with tile.TileContext(nc) as tc:
        with ExitStack() as ctx:
            emit_core_kernel(ctx, tc, io, n=n, dim=dim, hc=hc, dh=dh, qch=qch)
    nc.compile()
    return nc


def make_core_inputs(x_b, Wq, Wk, Wv, bq, bk, bv, Wout_g, n=2048, dim=512,
                     hc=4, dh=64):
    """Host-side prep of one core's input dict (numpy, correct dtypes)."""
    f16 = np.float16
    inner = hc * dh
    xh = x_b.astype(f16)
    xl = (x_b - xh.astype(np.float32)).astype(f16)
    wqk = np.concatenate([Wq, Wk], axis=1)              # [dim, 2*inner]
    wqk_hi = wqk.astype(f16)
    wqk_lo = (wqk - wqk_hi.astype(np.float32)).astype(f16)
    wqk_x = np.concatenate([wqk_lo, wqk_hi], axis=0)    # [2*dim, 2*inner]
    # per-head biases duplicated across both 64-row halves: [bq_h;bq_h] x hc
    # then [bk_h;bk_h] x hc
    bq_h = bq.reshape(hc, dh)
    bk_h = bk.reshape(hc, dh)
    bqk2 = np.concatenate([np.concatenate([bq_h[h], bq_h[h]]) for h in range(hc)]
                          + [np.concatenate([bk_h[h], bk_h[h]]) for h in range(hc)])
    return {
        "xt_h": np.ascontiguousarray(xh.T),
        "xt_l": np.ascontiguousarray(xl.T),
        "wqk_h": wqk_hi, "wqk_x": wqk_x,
        "wv_h": Wv.astype(f16),
        "bqk2": bqk2.reshape(2 * hc * 128, 1).astype(np.float32),
        "bv": bv.reshape(1, inner).astype(f16),
        "wout_b": Wout_g.astype(f16),
    }


@functools.lru_cache(maxsize=1)
def _cached_program():
    return build_program()


def kernel(x, Wqkv, bqkv, Wout, bout):
    x = np.asarray(x, dtype=np.float32)
    Wqkv = np.asarray(Wqkv, dtype=np.float32)
    bqkv = np.asarray(bqkv, dtype=np.float32)
    Wout = np.asarray(Wout, dtype=np.float32)
    bout = np.asarray(bout, dtype=np.float32)

    b, n, dim = x.shape
    H, dh = 8, 64
    inner = H * dh
    hc = 4  # heads per core
    Wq, Wk, Wv = Wqkv[:, :inner], Wqkv[:, inner:2 * inner], Wqkv[:, 2 * inner:]
    bq, bk, bv = bqkv[:inner], bqkv[inner:2 * inner], bqkv[2 * inner:]

    in_maps = []
    for c in range(8):
        bb, g = c // 2, c % 2
        hsl = slice(g * hc * dh, (g + 1) * hc * dh)
        in_maps.append(make_core_inputs(
            x[bb], Wq[:, hsl], Wk[:, hsl], Wv[:, hsl],
            bq[hsl], bk[hsl], bv[hsl], Wout[hsl, :],
            n=n, dim=dim, hc=hc, dh=dh))

    nc = _cached_program()
    res = bass_utils.run_bass_kernel_spmd(nc, in_maps, core_ids=list(range(8)))
    global LAST_RESULTS
    LAST_RESULTS = res
    out = np.empty((b, n, dim), dtype=np.float32)
    for bb in range(b):
        out[bb] = res.results[2 * bb]["out"] + res.results[2 * bb + 1]["out"] \
            + bout
    return out
